# revision 1
# baseline (speedup 1.0000x reference)
"""Differentiable Gaussian-splat tile compositor on 8 Trainium2 cores.

Strategy (sharding_hint): image split into 8 horizontal bands (32 rows each),
one band per NeuronCore. Within a band, 16x16 pixel tiles; each Gaussian is
assigned (host-side, exact per-pixel-center test) to the tiles it can touch
(alpha >= 1/255 <=> q <= 2*ln(255)). The 32 per-tile depth-ordered Gaussian
lists are packed 4-tiles-per-sweep into 8 sweeps of <=128 rows (segments =
tiles; the strict-lower block-diagonal mask that realizes the per-tile
exclusive cumulative sum of ln(1-alpha) is DMA'd as *data*, which keeps the
device program identical across cores = SPMD).

Device math per sweep (g = packed Gaussian rows, pix = 256 tile-local pixels):
  q[g,pix]   = A[12,g]^T @ B[12,pix]      (PE, f32r hi/lo split, exact basis)
  e          = exp(-q/2)                  (ACT)
  m          = e >= 1/255                 (DVE)
  alpha      = min(e,.99) * m             (DVE fused scalar_tensor_tensor)
  l          = ln(-alpha + 1)             (ACT, free affine does 1-alpha)
  Tlog[g,pix]= StrictLowerBlockDiag @ l   (PE, per-tile exclusive cumsum)
  T          = exp(Tlog)                  (ACT)
  w          = alpha * T                  (Pool)
  img[24 rows of group] = Colors^T @ w    (PE; each group of 2 sweeps = one
                                           8-tile quarter finalizing its own
                                           24 output rows -> per-group PSUM
                                           evacuation + output DMA overlap
                                           the remaining groups)
Host reassembles bands from the per-core row-base slot map.
"""

import os
import numpy as np

_H = 256
_W = 256
_NCORES = 8
_TS = 16                       # pixel tile edge
_TILES_X = _W // _TS           # 16
_TILES_Y_CORE = (_H // _NCORES) // _TS   # 2 tile rows per core band
_NTILES = _TILES_X * _TILES_Y_CORE       # 32 tiles per core
_NPIX = _TS * _TS              # 256 pixels per tile
_CAP = 128                     # gaussian rows per sweep
_S = 8                         # sweeps (32 tiles / 4 per sweep)
_SLOTS = 4                     # tiles per sweep
_GROUP = 2                     # sweeps batched per PSUM group
_QTH = float(2.0 * np.log(255.0))
_PAD_Q = 100.0                 # q for padding slots -> alpha 0


def _f32r_hi(x):
    """Truncate f32 mantissa to 10 explicit bits (safely representable in
    the PE's reduced-precision f32r streaming format)."""
    xi = np.ascontiguousarray(x, dtype=np.float32).view(np.int32)
    return (xi & np.int32(~0x1FFF)).view(np.float32)


def _reference_numpy(means_2d, covs_2d, depth_features, color_features, H, W):
    """Exact slow fallback (mirrors reference.py math)."""
    order = np.argsort(depth_features, kind="stable")
    m = means_2d[order].astype(np.float32)
    cv = covs_2d[order].astype(np.float32)
    cl = color_features[order].astype(np.float32)
    a, b, c = cv[:, 0], cv[:, 1], cv[:, 2]
    det = a * c - b * b
    ia, ib, ic = c / det, -b / det, a / det
    xs = np.arange(W, dtype=np.float32) + 0.5
    ys = np.arange(H, dtype=np.float32) + 0.5
    img = np.zeros((3, H, W), np.float32)
    T = np.ones((H, W), np.float32)
    for p in range(m.shape[0]):
        dx = xs[None, :] - m[p, 0]
        dy = ys[:, None] - m[p, 1]
        q = ia[p] * dx * dx + 2.0 * ib[p] * dx * dy + ic[p] * dy * dy
        alpha = np.minimum(np.float32(0.99), np.exp(np.float32(-0.5) * q))
        alpha = np.where(alpha < 1.0 / 255.0, np.float32(0.0), alpha)
        w = alpha * T
        img += cl[p][:, None, None] * w[None]
        T = T * (1.0 - alpha)
    return img


def _prep_core(core, m, ia, ib, ic, rx, ry):
    """Per-tile depth-ordered gaussian lists for one core band."""
    tiles = []
    y_base = core * (_H // _NCORES)
    for ty in range(_TILES_Y_CORE):
        y0 = y_base + ty * _TS
        for tx in range(_TILES_X):
            x0 = tx * _TS
            t = ty * _TILES_X + tx
            cand = np.nonzero(
                (m[:, 0] + rx >= x0 + 0.5 - 1e-6)
                & (m[:, 0] - rx <= x0 + _TS - 0.5 + 1e-6)
                & (m[:, 1] + ry >= y0 + 0.5 - 1e-6)
                & (m[:, 1] - ry <= y0 + _TS - 0.5 + 1e-6)
            )[0]
            if cand.size:
                # exact: min over the tile's pixel centers of q <= QTH
                dx = (x0 + 0.5 + np.arange(_TS))[None, :] - m[cand, 0][:, None]
                dy = (y0 + 0.5 + np.arange(_TS))[None, :] - m[cand, 1][:, None]
                q = (
                    ia[cand][:, None, None] * (dx * dx)[:, None, :]
                    + 2.0 * ib[cand][:, None, None]
                    * dx[:, None, :] * dy[:, :, None]
                    + ic[cand][:, None, None] * (dy * dy)[:, :, None]
                )
                qmin = q.reshape(cand.size, -1).min(axis=1)
                cand = cand[qmin <= _QTH + 1e-3]
            tiles.append((t, cand))
    return tiles


def _pack_tiles(tiles):
    """Pack the 32 tiles into _S sweeps of _SLOTS tiles. Tiles are first
    balanced into 4 quarters of 8 tiles (quarter q -> sweeps 2q, 2q+1 =
    PSUM group q, which finalizes its own 24 image rows independently),
    then each quarter splits into 2 sweeps of 4 tiles, each <= _CAP rows.
    Returns sweeps: list of _S lists of (tile, idx, start_row)."""
    order = sorted(range(len(tiles)), key=lambda i: -len(tiles[i][1]))
    quarters = [[] for _ in range(4)]
    qload = [0] * 4
    for i in order:
        cands = [q for q in range(4) if len(quarters[q]) < 8]
        q = min(cands, key=lambda q: qload[q])
        quarters[q].append(i)
        qload[q] += len(tiles[i][1])
    sweeps = [[] for _ in range(_S)]
    loads = [0] * _S
    for q, members in enumerate(quarters):
        for i in sorted(members, key=lambda i: -len(tiles[i][1])):
            t, idx = tiles[i]
            n = len(idx)
            cands = [
                sw for sw in (2 * q, 2 * q + 1) if len(sweeps[sw]) < _SLOTS
            ]
            sw = min(cands, key=lambda sw: loads[sw])
            if loads[sw] + n > _CAP:
                raise ValueError(f"sweep overflow: {loads[sw]}+{n} > {_CAP}")
            sweeps[sw].append((t, idx, loads[sw]))
            loads[sw] += n
    return sweeps


def _build_core_data(core, m, ia, ib, ic, cl, rx, ry):
    """Host tensors for one core: A [12, S*128], mask [128, S*128],
    colors [128, S*12] (f32), and slotmap tile -> (sweep, slot)."""
    tiles = _prep_core(core, m, ia, ib, ic, rx, ry)
    sweeps = _pack_tiles(tiles)

    A = np.zeros((12, _S * _CAP), np.float32)
    A[5, :] = _PAD_Q            # padding slots: q == _PAD_Q everywhere
    mask = np.zeros((128, _S * _CAP), np.float32)
    colors = np.zeros((128, _S * 24), np.float32)
    slotmap = [None] * _NTILES

    y_base = core * (_H // _NCORES)
    for s, sw in enumerate(sweeps):
        for j, (t, idx, start) in enumerate(sw):
            slotmap[t] = 24 * (s // 2) + 12 * (s % 2) + 3 * j
            n = len(idx)
            if n == 0:
                continue
            ty, tx = divmod(t, _TILES_X)
            cx = tx * _TS + 8.0            # tile-local frame origin
            cy = y_base + ty * _TS + 8.0
            mxl = m[idx, 0] - cx
            myl = m[idx, 1] - cy
            g_ia, g_ib, g_ic = ia[idx], ib[idx], ic[idx]
            coef = np.stack(
                [
                    g_ia,
                    2.0 * g_ib,
                    g_ic,
                    -2.0 * (g_ia * mxl + g_ib * myl),
                    -2.0 * (g_ib * mxl + g_ic * myl),
                    g_ia * mxl * mxl + 2.0 * g_ib * mxl * myl
                    + g_ic * myl * myl,
                ],
                axis=0,
            )  # [6, n] float64
            hi = _f32r_hi(coef.astype(np.float32))
            lo = (coef - hi.astype(np.float64)).astype(np.float32)
            c0 = s * _CAP + start
            A[:6, c0 : c0 + n] = hi
            A[6:, c0 : c0 + n] = lo
            # mask[row g', col g] = 1 iff g' < g within the segment
            mask[start : start + n, c0 : c0 + n] = np.triu(
                np.ones((n, n), np.float32), 1
            )
            cc = s * 24 + 12 * (s % 2) + 3 * j
            colors[start : start + n, cc : cc + 3] = cl[idx]
    return A, mask, colors, slotmap


def _basis():
    lc = np.arange(_TS, dtype=np.float32) - 7.5
    xl = np.tile(lc, _TS)                     # pixel p = lr*16+lc
    yl = np.repeat(lc, _TS)
    B = np.stack(
        [xl * xl, xl * yl, yl * yl, xl, yl, np.ones(_NPIX, np.float32)], 0
    )
    return np.concatenate([B, B], axis=0).astype(np.float32)   # [12, 256]


def _build_program(reps=1):
    from contextlib import ExitStack

    import concourse.bacc as bacc
    import concourse.hw_specs as hw_specs
    import concourse.tile as tile
    from concourse import mybir

    F32 = mybir.dt.float32
    F32R = mybir.dt.float32r
    AF = mybir.ActivationFunctionType
    OP = mybir.AluOpType
    S = _S

    # Our kernel alternates Exp and Ln; make sure the act-table chooser can
    # only satisfy both from the combined set (one table load instead of a
    # ~1.3us reload per switch). Keys and their order are preserved so the
    # emitted act_func_set_id indices stay aligned with act_info.json.
    if not getattr(hw_specs, "_gs_act_patch", False):
        _orig_get_tables = hw_specs.get_activation_tables

        def _patched(arch):
            tables = _orig_get_tables(arch)
            for name, funcs in tables.items():
                if name != "natural_log_exp_and_others":
                    funcs.discard(mybir.ActivationFunctionType.Exp)
                    funcs.discard(mybir.ActivationFunctionType.Ln)
            return tables

        hw_specs.get_activation_tables = _patched
        bacc.get_activation_tables = _patched
        hw_specs._gs_act_patch = True

    nc = bacc.Bacc(trn_type="TRN2", target_bir_lowering=False, debug=False)
    t_A = nc.dram_tensor(
        "A", [12, _NPIX + S * _CAP], F32, kind="ExternalInput"
    )  # basis in cols [0, _NPIX), coefficients after
    t_mask = nc.dram_tensor("maskl", [128, S * _CAP], F32, kind="ExternalInput")
    t_col = nc.dram_tensor(
        "colors", [128, S * 24], F32, kind="ExternalInput"
    )
    t_out = nc.dram_tensor(
        "out", [3 * _NTILES, _NPIX], F32, kind="ExternalOutput"
    )

    NG = S // _GROUP           # number of groups
    GW = _GROUP * _NPIX        # group free width
    ROWS_G = 3 * _SLOTS * _GROUP   # img rows finalized per group

    with ExitStack() as ctx:
        tc = ctx.enter_context(tile.TileContext(nc))
        const = ctx.enter_context(tc.tile_pool(name="const", bufs=1))
        sb = ctx.enter_context(tc.tile_pool(name="sb", bufs=4))
        psq = ctx.enter_context(tc.tile_pool(name="psq", bufs=2, space="PSUM"))
        pst = ctx.enter_context(tc.tile_pool(name="pst", bufs=2, space="PSUM"))
        psi = ctx.enter_context(tc.tile_pool(name="psi", bufs=2, space="PSUM"))

        AB_all = const.tile([12, _NPIX + S * _CAP], F32)
        mask_all = const.tile([128, S * _CAP], F32)
        col_all = const.tile([128, S * 24], F32)

        # basis + A coefficients on the SP queue (gate the q matmuls;
        # chunk 0 carries basis + group-0 coeffs in one DMA so only one
        # DMA completion latency sits before the first matmul);
        # masks + colors on the gpsimd queue in parallel.
        CW = _GROUP * _CAP
        nc.gpsimd.dma_start(
            AB_all[:, : _NPIX + CW].bitcast(F32R),
            t_A[:, : _NPIX + CW].bitcast(F32R),
        )
        for g in range(1, NG):
            c0 = _NPIX + g * CW
            nc.sync.dma_start(
                AB_all[:, c0 : c0 + CW].bitcast(F32R),
                t_A[:, c0 : c0 + CW].bitcast(F32R),
            )
        for g in range(NG):
            nc.gpsimd.dma_start(
                mask_all[:, g * CW : (g + 1) * CW].bitcast(F32R),
                t_mask[:, g * CW : (g + 1) * CW].bitcast(F32R),
            )
        nc.gpsimd.dma_start(col_all[:].bitcast(F32R), t_col[:].bitcast(F32R))

        basis = AB_all[:, :_NPIX]
        A_t = [
            AB_all[:, _NPIX + s * _CAP : _NPIX + (s + 1) * _CAP]
            for s in range(S)
        ]
        mask_t = [mask_all[:, s * _CAP : (s + 1) * _CAP] for s in range(S)]
        col_t = [col_all[:, s * 24 : (s + 1) * 24] for s in range(S)]

        # warm the PE clock (HAM) while input DMAs are in flight
        psw = ctx.enter_context(tc.tile_pool(name="psw", bufs=1, space="PSUM"))
        warm = const.tile([128, 16], F32)
        nc.vector.memset(warm[:], 0.0)
        warm_ps = psw.tile([128, 16], F32)
        for _ in range(14):
            nc.tensor.matmul(
                warm_ps[:16, :16], warm[:], warm[:, :16], start=True, stop=True
            )

        for g in range(NG * reps):
            g = g % NG
            q4 = psq.tile([128, GW], F32)
            for i in range(_GROUP):
                s = g * _GROUP + i
                nc.tensor.matmul(
                    q4[:, i * _NPIX : (i + 1) * _NPIX],
                    A_t[s].bitcast(F32R),
                    basis.bitcast(F32R),
                    start=True,
                    stop=True,
                )
            e4 = sb.tile([128, GW], F32, tag="e")
            nc.scalar.activation(e4[:], q4[:], AF.Exp, scale=-0.5)
            m4 = sb.tile([128, GW], F32, tag="m")
            nc.vector.tensor_scalar(m4[:], e4[:], 1.0 / 255.0, None, OP.is_ge)
            al4 = sb.tile([128, GW], F32, tag="al")
            nc.vector.scalar_tensor_tensor(
                al4[:], e4[:], 0.99, m4[:], OP.min, OP.mult
            )
            l4 = sb.tile([128, GW], F32, tag="l")
            nc.scalar.activation(
                l4[:].bitcast(F32R), al4[:], AF.Ln, bias=1.0, scale=-1.0
            )
            tl4 = pst.tile([128, GW], F32)
            for i in range(_GROUP):
                s = g * _GROUP + i
                nc.tensor.matmul(
                    tl4[:, i * _NPIX : (i + 1) * _NPIX],
                    mask_t[s].bitcast(F32R),
                    l4[:, i * _NPIX : (i + 1) * _NPIX].bitcast(F32R),
                    start=True,
                    stop=True,
                )
            T4 = sb.tile([128, GW], F32, tag="T")
            nc.scalar.activation(T4[:], tl4[:], AF.Exp)
            w4 = sb.tile([128, GW], F32, tag="w")
            nc.gpsimd.tensor_tensor(
                w4[:].bitcast(F32R), al4[:], T4[:], OP.mult
            )
            img = psi.tile([24, _NPIX], F32, tag="img", name="img")
            for i in range(_GROUP):
                s = g * _GROUP + i
                nc.tensor.matmul(
                    img[:],
                    col_t[s].bitcast(F32R),
                    w4[:, i * _NPIX : (i + 1) * _NPIX].bitcast(F32R),
                    start=(i == 0),
                    stop=(i == _GROUP - 1),
                )
            out_sb = sb.tile([24, _NPIX], F32, tag="osb", name="osb")
            nc.vector.tensor_copy(out_sb[:], img[:])
            nc.sync.dma_start(t_out[g * 24 : (g + 1) * 24, :], out_sb[:])

    nc.compile()
    return nc


def kernel(means_2d, covs_2d, depth_features, color_features, height, width):
    H, W = int(height), int(width)
    means_2d = np.asarray(means_2d, np.float32)
    covs_2d = np.asarray(covs_2d, np.float32)
    depth_features = np.asarray(depth_features, np.float32)
    color_features = np.asarray(color_features, np.float32)

    a, b, c = (
        covs_2d[:, 0].astype(np.float64),
        covs_2d[:, 1].astype(np.float64),
        covs_2d[:, 2].astype(np.float64),
    )
    det = a * c - b * b
    if H != _H or W != _W or np.any(det <= 0) or np.any(a <= 0) or np.any(c <= 0):
        return _reference_numpy(
            means_2d, covs_2d, depth_features, color_features, H, W
        )

    order = np.argsort(depth_features, kind="stable")
    m = means_2d[order].astype(np.float64)
    cvo = covs_2d[order].astype(np.float64)
    cl = color_features[order].astype(np.float32)
    a, b, c = cvo[:, 0], cvo[:, 1], cvo[:, 2]
    det = a * c - b * b
    ia, ib, ic = c / det, -b / det, a / det
    rx = np.sqrt(_QTH * a) + 1e-3
    ry = np.sqrt(_QTH * c) + 1e-3

    try:
        in_maps = []
        slotmaps = []
        basis = _basis()
        for core in range(_NCORES):
            A, mask, colors, slotmap = _build_core_data(
                core, m, ia, ib, ic, cl, rx, ry
            )
            in_maps.append(
                {
                    "A": np.ascontiguousarray(
                        np.concatenate([basis, A], axis=1)
                    ),
                    "maskl": mask,
                    "colors": colors,
                }
            )
            slotmaps.append(slotmap)
    except ValueError:
        return _reference_numpy(
            means_2d, covs_2d, depth_features, color_features, H, W
        )

    nc = _build_program()
    if os.environ.get("GS_KERNEL_SIM") == "1":
        from types import SimpleNamespace

        from concourse.bass_interp import CoreSim

        results = []
        for core in range(_NCORES):
            sim = CoreSim(nc)
            for k, v in in_maps[core].items():
                sim.tensor(k)[:] = v
            sim.simulate()
            results.append({"out": np.array(sim.tensor("out"))})
        res = SimpleNamespace(results=results)
    else:
        from concourse.bass_utils import run_bass_kernel_spmd

        res = run_bass_kernel_spmd(nc, in_maps, core_ids=list(range(_NCORES)))

    img = np.zeros((3, _H, _W), np.float32)
    band = _H // _NCORES
    for core in range(_NCORES):
        o = res.results[core]["out"]  # [96, 256]
        rowbase = slotmaps[core]
        for t in range(_NTILES):
            ty, tx = divmod(t, _TILES_X)
            blk = o[rowbase[t] : rowbase[t] + 3].reshape(3, _TS, _TS)
            img[
                :,
                core * band + ty * _TS : core * band + (ty + 1) * _TS,
                tx * _TS : (tx + 1) * _TS,
            ] = blk
    return img



# revision 4
# speedup vs baseline: 1.1419x; 1.1419x over previous
"""Differentiable Gaussian-splat tile compositor on 8 Trainium2 cores.

Strategy (sharding_hint): image split into 8 horizontal bands (32 rows each),
one band per NeuronCore. Within a band, 16x16 pixel tiles; each Gaussian is
assigned (host-side, exact per-pixel-center test) to the tiles it can touch
(alpha >= 1/255 <=> q <= 2*ln(255)). The per-tile depth-ordered Gaussian
lists are packed into 6 sweeps of exactly <=128 rows each; a tile's list may
SPLIT across two consecutive sweeps — the exclusive cumulative transmittance
of the second part picks up the first part's tail product via an extra
"carry" matmul in log space (carry matrix DMA'd as data, so the device
program stays identical across cores = SPMD).

Device math per sweep (g = packed Gaussian rows, pix = 256 tile-local pixels):
  q[g,pix]   = A[12,g]^T @ B[12,pix]      (PE, f32r hi/lo split, exact basis)
  e          = exp(-q/2)                  (ACT)
  m          = e >= 1/255                 (DVE)
  alpha      = min(e,.99) * m             (DVE fused scalar_tensor_tensor)
  l          = ln(-alpha + 1)             (ACT, free affine does 1-alpha)
  Tlog[g,pix]= StrictLowerBlockDiag @ l   (PE, per-part exclusive cumsum)
               + Carry_s @ l_prev         (PE, split-tile cross-sweep carry)
  T          = exp(Tlog)                  (ACT)
  w          = alpha * T                  (Pool)
  img[group rows] = Colors^T @ w          (PE accumulate over the group's 2
                                           sweeps; DMA'd straight from PSUM)
Host reassembles bands from the per-core (tile, group, row) slot map, adding
the partial sums of split tiles.
"""

import os
import numpy as np

_H = 256
_W = 256
_NCORES = 8
_TS = 16                       # pixel tile edge
_TILES_X = _W // _TS           # 16
_TILES_Y_CORE = (_H // _NCORES) // _TS   # 2 tile rows per core band
_NTILES = _TILES_X * _TILES_Y_CORE       # 32 tiles per core
_NPIX = _TS * _TS              # 256 pixels per tile
_CAP = 128                     # gaussian rows per sweep
_S = 6                         # sweeps (perfect-fill packing w/ splits)
_GROUP = 2                     # sweeps batched per PSUM group
_NG = _S // _GROUP             # groups
_QTH = float(2.0 * np.log(255.0))
_PAD_Q = 100.0                 # q for padding slots -> alpha 0


def _f32r_hi(x):
    """Truncate f32 mantissa to 10 explicit bits (safely representable in
    the PE's reduced-precision f32r streaming format)."""
    xi = np.ascontiguousarray(x, dtype=np.float32).view(np.int32)
    return (xi & np.int32(~0x1FFF)).view(np.float32)


def _reference_numpy(means_2d, covs_2d, depth_features, color_features, H, W):
    """Exact slow fallback (mirrors reference.py math)."""
    order = np.argsort(depth_features, kind="stable")
    m = means_2d[order].astype(np.float32)
    cv = covs_2d[order].astype(np.float32)
    cl = color_features[order].astype(np.float32)
    a, b, c = cv[:, 0], cv[:, 1], cv[:, 2]
    det = a * c - b * b
    ia, ib, ic = c / det, -b / det, a / det
    xs = np.arange(W, dtype=np.float32) + 0.5
    ys = np.arange(H, dtype=np.float32) + 0.5
    img = np.zeros((3, H, W), np.float32)
    T = np.ones((H, W), np.float32)
    for p in range(m.shape[0]):
        dx = xs[None, :] - m[p, 0]
        dy = ys[:, None] - m[p, 1]
        q = ia[p] * dx * dx + 2.0 * ib[p] * dx * dy + ic[p] * dy * dy
        alpha = np.minimum(np.float32(0.99), np.exp(np.float32(-0.5) * q))
        alpha = np.where(alpha < 1.0 / 255.0, np.float32(0.0), alpha)
        w = alpha * T
        img += cl[p][:, None, None] * w[None]
        T = T * (1.0 - alpha)
    return img


def _prep_core(core, m, ia, ib, ic, rx, ry):
    """Per-tile depth-ordered gaussian lists for one core band."""
    tiles = []
    y_base = core * (_H // _NCORES)
    for ty in range(_TILES_Y_CORE):
        y0 = y_base + ty * _TS
        for tx in range(_TILES_X):
            x0 = tx * _TS
            t = ty * _TILES_X + tx
            cand = np.nonzero(
                (m[:, 0] + rx >= x0 + 0.5 - 1e-6)
                & (m[:, 0] - rx <= x0 + _TS - 0.5 + 1e-6)
                & (m[:, 1] + ry >= y0 + 0.5 - 1e-6)
                & (m[:, 1] - ry <= y0 + _TS - 0.5 + 1e-6)
            )[0]
            if cand.size:
                # exact: min over the tile's pixel centers of q <= QTH
                dx = (x0 + 0.5 + np.arange(_TS))[None, :] - m[cand, 0][:, None]
                dy = (y0 + 0.5 + np.arange(_TS))[None, :] - m[cand, 1][:, None]
                q = (
                    ia[cand][:, None, None] * (dx * dx)[:, None, :]
                    + 2.0 * ib[cand][:, None, None]
                    * dx[:, None, :] * dy[:, :, None]
                    + ic[cand][:, None, None] * (dy * dy)[:, :, None]
                )
                qmin = q.reshape(cand.size, -1).min(axis=1)
                cand = cand[qmin <= _QTH + 1e-3]
            tiles.append((t, cand))
    return tiles


def _pack_tiles(tiles):
    """Sequentially fill _S sweeps of exactly _CAP rows, splitting a tile's
    depth-ordered list across two consecutive sweeps when it straddles a
    boundary. Returns parts: list of (tile, sweep, start_row, idx_chunk,
    carry_from) where carry_from = (prev_sweep_start, prev_len) for the
    second half of a split, else None."""
    parts = []
    sweep, cursor = 0, 0
    for t, idx in tiles:
        n = len(idx)
        if n == 0:
            continue
        off = 0
        prev = None
        while n > 0:
            if cursor == _CAP:
                sweep += 1
                cursor = 0
                if sweep >= _S:
                    raise ValueError("packing overflow")
            take = min(n, _CAP - cursor)
            parts.append((t, sweep, cursor, idx[off : off + take], prev))
            prev = (cursor, take)   # for a possible continuation
            cursor += take
            off += take
            n -= take
    return parts


def _build_core_data(core, m, ia, ib, ic, cl, rx, ry, rows_g):
    """Host tensors for one core: A [12, S*128], mask [128, S*128],
    carry [128, (S-1)*128], colors [128, S*rows_g], and slotmap entries
    (tile, group, rowbase)."""
    tiles = _prep_core(core, m, ia, ib, ic, rx, ry)
    parts = _pack_tiles(tiles)

    A = np.zeros((12, _S * _CAP), np.float32)
    A[5, :] = _PAD_Q            # padding slots: q == _PAD_Q everywhere
    mask = np.zeros((128, _S * _CAP), np.float32)
    carry = np.zeros((128, (_S - 1) * _CAP), np.float32)
    colors = np.zeros((128, _S * rows_g), np.float32)
    slotmap = []

    # assign output rows per group: parts in sweeps 2g, 2g+1 get 3 rows each
    grow = [0] * _NG
    y_base = core * (_H // _NCORES)
    for t, s, start, idx, prev in parts:
        g = s // _GROUP
        rowbase = grow[g]
        grow[g] += 3
        if grow[g] > rows_g:
            raise ValueError("group row overflow")
        slotmap.append((t, g, rowbase))
        n = len(idx)
        ty, tx = divmod(t, _TILES_X)
        cx = tx * _TS + 8.0            # tile-local frame origin
        cy = y_base + ty * _TS + 8.0
        mxl = m[idx, 0] - cx
        myl = m[idx, 1] - cy
        g_ia, g_ib, g_ic = ia[idx], ib[idx], ic[idx]
        coef = np.stack(
            [
                g_ia,
                2.0 * g_ib,
                g_ic,
                -2.0 * (g_ia * mxl + g_ib * myl),
                -2.0 * (g_ib * mxl + g_ic * myl),
                g_ia * mxl * mxl + 2.0 * g_ib * mxl * myl
                + g_ic * myl * myl,
            ],
            axis=0,
        )  # [6, n] float64
        hi = _f32r_hi(coef.astype(np.float32))
        lo = (coef - hi.astype(np.float64)).astype(np.float32)
        c0 = s * _CAP + start
        A[:6, c0 : c0 + n] = hi
        A[6:, c0 : c0 + n] = lo
        # mask[row g', col g] = 1 iff g' < g within the part
        mask[start : start + n, c0 : c0 + n] = np.triu(
            np.ones((n, n), np.float32), 1
        )
        if prev is not None:
            # continuation: add the whole part-1 log-sum to every row
            ps, pn = prev
            carry[ps : ps + pn, (s - 1) * _CAP + start : (s - 1) * _CAP + start + n] = 1.0
        colors[start : start + n, s * rows_g + rowbase : s * rows_g + rowbase + 3] = cl[idx]
    return A, mask, carry, colors, slotmap


def _count_rows_g(m, ia, ib, ic, rx, ry):
    """Max output rows needed by any group on any core (3 per tile-part)."""
    worst = 0
    for core in range(_NCORES):
        parts = _pack_tiles(_prep_core(core, m, ia, ib, ic, rx, ry))
        grow = [0] * _NG
        for t, s, start, idx, prev in parts:
            grow[s // _GROUP] += 3
        worst = max(worst, max(grow))
    return worst


def _basis():
    lc = np.arange(_TS, dtype=np.float32) - 7.5
    xl = np.tile(lc, _TS)                     # pixel p = lr*16+lc
    yl = np.repeat(lc, _TS)
    B = np.stack(
        [xl * xl, xl * yl, yl * yl, xl, yl, np.ones(_NPIX, np.float32)], 0
    )
    return np.concatenate([B, B], axis=0).astype(np.float32)   # [12, 256]


def _build_program(rows_g, reps=1):
    from contextlib import ExitStack

    import concourse.bacc as bacc
    import concourse.hw_specs as hw_specs
    import concourse.tile as tile
    from concourse import mybir

    F32 = mybir.dt.float32
    F32R = mybir.dt.float32r
    AF = mybir.ActivationFunctionType
    OP = mybir.AluOpType
    S = _S

    # Our kernel alternates Exp and Ln; make sure the act-table chooser can
    # only satisfy both from the combined set (one table load instead of a
    # ~1.3us reload per switch). Keys and their order are preserved so the
    # emitted act_func_set_id indices stay aligned with act_info.json.
    if not getattr(hw_specs, "_gs_act_patch", False):
        _orig_get_tables = hw_specs.get_activation_tables

        def _patched(arch):
            tables = _orig_get_tables(arch)
            for name, funcs in tables.items():
                if name != "natural_log_exp_and_others":
                    funcs.discard(mybir.ActivationFunctionType.Exp)
                    funcs.discard(mybir.ActivationFunctionType.Ln)
            return tables

        hw_specs.get_activation_tables = _patched
        bacc.get_activation_tables = _patched
        hw_specs._gs_act_patch = True

    nc = bacc.Bacc(trn_type="TRN2", target_bir_lowering=False, debug=False)
    t_A = nc.dram_tensor(
        "A", [12, _NPIX + S * _CAP], F32, kind="ExternalInput"
    )  # basis in cols [0, _NPIX), coefficients after
    t_mask = nc.dram_tensor("maskl", [128, S * _CAP], F32, kind="ExternalInput")
    t_carry = nc.dram_tensor(
        "carry", [128, (S - 1) * _CAP], F32, kind="ExternalInput"
    )
    t_col = nc.dram_tensor(
        "colors", [128, S * rows_g], F32, kind="ExternalInput"
    )
    t_out = nc.dram_tensor(
        "out", [_NG * rows_g, _NPIX], F32, kind="ExternalOutput"
    )

    GW = _GROUP * _NPIX        # group free width

    with ExitStack() as ctx:
        tc = ctx.enter_context(tile.TileContext(nc))
        const = ctx.enter_context(tc.tile_pool(name="const", bufs=1))
        sb = ctx.enter_context(tc.tile_pool(name="sb", bufs=4))
        psq = ctx.enter_context(tc.tile_pool(name="psq", bufs=2, space="PSUM"))
        pst = ctx.enter_context(tc.tile_pool(name="pst", bufs=2, space="PSUM"))
        psi = ctx.enter_context(tc.tile_pool(name="psi", bufs=2, space="PSUM"))

        AB_all = const.tile([12, _NPIX + S * _CAP], F32)
        mask_all = const.tile([128, S * _CAP], F32)
        carry_all = const.tile([128, (S - 1) * _CAP], F32)
        col_all = const.tile([128, S * rows_g], F32)

        # basis + A coefficients on the SP queue (gate the q matmuls;
        # chunk 0 carries basis + group-0 coeffs in one DMA so only one
        # DMA completion latency sits before the first matmul);
        # masks on the gpsimd queue in parallel; carries on the vector
        # queue (first chunk alone so sweep-1's carry lands early).
        CW = _GROUP * _CAP
        nc.gpsimd.dma_start(
            AB_all[:, : _NPIX + CW].bitcast(F32R),
            t_A[:, : _NPIX + CW].bitcast(F32R),
        )
        c0 = _NPIX + CW
        nc.sync.dma_start(
            AB_all[:, c0 : c0 + CW].bitcast(F32R),
            t_A[:, c0 : c0 + CW].bitcast(F32R),
        )
        nc.sync.dma_start(
            carry_all[:, :_CAP].bitcast(F32R), t_carry[:, :_CAP].bitcast(F32R)
        )
        c0 = _NPIX + 2 * CW
        nc.sync.dma_start(
            AB_all[:, c0 : c0 + CW].bitcast(F32R),
            t_A[:, c0 : c0 + CW].bitcast(F32R),
        )
        nc.sync.dma_start(
            carry_all[:, _CAP:].bitcast(F32R), t_carry[:, _CAP:].bitcast(F32R)
        )
        for g in range(_NG):
            nc.gpsimd.dma_start(
                mask_all[:, g * CW : (g + 1) * CW].bitcast(F32R),
                t_mask[:, g * CW : (g + 1) * CW].bitcast(F32R),
            )
        nc.gpsimd.dma_start(col_all[:].bitcast(F32R), t_col[:].bitcast(F32R))

        basis = AB_all[:, :_NPIX]
        A_t = [
            AB_all[:, _NPIX + s * _CAP : _NPIX + (s + 1) * _CAP]
            for s in range(S)
        ]
        mask_t = [mask_all[:, s * _CAP : (s + 1) * _CAP] for s in range(S)]
        carry_t = [
            carry_all[:, s * _CAP : (s + 1) * _CAP] for s in range(S - 1)
        ]
        col_t = [col_all[:, s * rows_g : (s + 1) * rows_g] for s in range(S)]

        # warm the PE clock (HAM) while input DMAs are in flight
        psw = ctx.enter_context(tc.tile_pool(name="psw", bufs=1, space="PSUM"))
        warm = const.tile([128, 16], F32)
        nc.vector.memset(warm[:], 0.0)
        warm_ps = psw.tile([128, 16], F32)
        for _ in range(14):
            nc.tensor.matmul(
                warm_ps[:16, :16], warm[:], warm[:, :16], start=True, stop=True
            )

        l_tiles = {}
        for g in range(_NG * reps):
            g = g % _NG
            q4 = psq.tile([128, GW], F32)
            for i in range(_GROUP):
                s = g * _GROUP + i
                nc.tensor.matmul(
                    q4[:, i * _NPIX : (i + 1) * _NPIX],
                    A_t[s].bitcast(F32R),
                    basis.bitcast(F32R),
                    start=True,
                    stop=True,
                )
            e4 = sb.tile([128, GW], F32, tag="e")
            nc.scalar.activation(e4[:], q4[:], AF.Exp, scale=-0.5)
            m4 = sb.tile([128, GW], F32, tag="m")
            nc.vector.tensor_scalar(m4[:], e4[:], 1.0 / 255.0, None, OP.is_ge)
            al4 = sb.tile([128, GW], F32, tag="al")
            nc.vector.scalar_tensor_tensor(
                al4[:], e4[:], 0.99, m4[:], OP.min, OP.mult
            )
            l4 = sb.tile([128, GW], F32, tag=f"l{g % 2}")
            l_tiles[g] = l4
            nc.scalar.activation(
                l4[:].bitcast(F32R), al4[:], AF.Ln, bias=1.0, scale=-1.0
            )
            tl4 = pst.tile([128, GW], F32)
            for i in range(_GROUP):
                s = g * _GROUP + i
                nc.tensor.matmul(
                    tl4[:, i * _NPIX : (i + 1) * _NPIX],
                    mask_t[s].bitcast(F32R),
                    l4[:, i * _NPIX : (i + 1) * _NPIX].bitcast(F32R),
                    start=True,
                    stop=(s == 0),
                )
                if s > 0:
                    lprev = (
                        l4[:, :_NPIX]
                        if i == 1
                        else l_tiles[g - 1][:, _NPIX:]
                    )
                    nc.tensor.matmul(
                        tl4[:, i * _NPIX : (i + 1) * _NPIX],
                        carry_t[s - 1].bitcast(F32R),
                        lprev.bitcast(F32R),
                        start=False,
                        stop=True,
                    )
            T4 = sb.tile([128, GW], F32, tag="T")
            nc.scalar.activation(T4[:], tl4[:], AF.Exp)
            w4 = sb.tile([128, GW], F32, tag="w")
            nc.gpsimd.tensor_tensor(
                w4[:].bitcast(F32R), al4[:], T4[:], OP.mult
            )
            img = psi.tile([rows_g, _NPIX], F32, tag="img", name="img")
            for i in range(_GROUP):
                s = g * _GROUP + i
                nc.tensor.matmul(
                    img[:],
                    col_t[s].bitcast(F32R),
                    w4[:, i * _NPIX : (i + 1) * _NPIX].bitcast(F32R),
                    start=(i == 0),
                    stop=(i == _GROUP - 1),
                )
            out_sb = sb.tile([rows_g, _NPIX], F32, tag="osb", name="osb")
            nc.vector.tensor_copy(out_sb[:], img[:])
            nc.sync.dma_start(
                t_out[g * rows_g : (g + 1) * rows_g, :], out_sb[:]
            )

    nc.compile()
    return nc


def kernel(means_2d, covs_2d, depth_features, color_features, height, width):
    H, W = int(height), int(width)
    means_2d = np.asarray(means_2d, np.float32)
    covs_2d = np.asarray(covs_2d, np.float32)
    depth_features = np.asarray(depth_features, np.float32)
    color_features = np.asarray(color_features, np.float32)

    a, b, c = (
        covs_2d[:, 0].astype(np.float64),
        covs_2d[:, 1].astype(np.float64),
        covs_2d[:, 2].astype(np.float64),
    )
    det = a * c - b * b
    if H != _H or W != _W or np.any(det <= 0) or np.any(a <= 0) or np.any(c <= 0):
        return _reference_numpy(
            means_2d, covs_2d, depth_features, color_features, H, W
        )

    order = np.argsort(depth_features, kind="stable")
    m = means_2d[order].astype(np.float64)
    cvo = covs_2d[order].astype(np.float64)
    cl = color_features[order].astype(np.float32)
    a, b, c = cvo[:, 0], cvo[:, 1], cvo[:, 2]
    det = a * c - b * b
    ia, ib, ic = c / det, -b / det, a / det
    rx = np.sqrt(_QTH * a) + 1e-3
    ry = np.sqrt(_QTH * c) + 1e-3

    try:
        rows_g = _count_rows_g(m, ia, ib, ic, rx, ry)
        in_maps = []
        slotmaps = []
        basis = _basis()
        for core in range(_NCORES):
            A, mask, carry, colors, slotmap = _build_core_data(
                core, m, ia, ib, ic, cl, rx, ry, rows_g
            )
            in_maps.append(
                {
                    "A": np.ascontiguousarray(
                        np.concatenate([basis, A], axis=1)
                    ),
                    "maskl": mask,
                    "carry": carry,
                    "colors": colors,
                }
            )
            slotmaps.append(slotmap)
    except ValueError:
        return _reference_numpy(
            means_2d, covs_2d, depth_features, color_features, H, W
        )

    nc = _build_program(rows_g)
    if os.environ.get("GS_KERNEL_SIM") == "1":
        from types import SimpleNamespace

        from concourse.bass_interp import CoreSim

        results = []
        for core in range(_NCORES):
            sim = CoreSim(nc)
            for k, v in in_maps[core].items():
                sim.tensor(k)[:] = v
            sim.simulate()
            results.append({"out": np.array(sim.tensor("out"))})
        res = SimpleNamespace(results=results)
    else:
        from concourse.bass_utils import run_bass_kernel_spmd

        res = run_bass_kernel_spmd(nc, in_maps, core_ids=list(range(_NCORES)))

    img = np.zeros((3, _H, _W), np.float32)
    band = _H // _NCORES
    for core in range(_NCORES):
        o = res.results[core]["out"]  # [_NG*rows_g, 256]
        for t, g, rowbase in slotmaps[core]:
            ty, tx = divmod(t, _TILES_X)
            blk = o[g * rows_g + rowbase : g * rows_g + rowbase + 3].reshape(
                3, _TS, _TS
            )
            img[
                :,
                core * band + ty * _TS : core * band + (ty + 1) * _TS,
                tx * _TS : (tx + 1) * _TS,
            ] += blk
    return img


# revision 9
# speedup vs baseline: 1.1526x; 1.0094x over previous
"""Differentiable Gaussian-splat tile compositor on 8 Trainium2 cores.

Strategy (sharding_hint): image split into 8 horizontal bands (32 rows each),
one band per NeuronCore. Within a band, 16x16 pixel tiles; each Gaussian is
assigned (host-side, exact per-pixel-center test) to the tiles it can touch
(alpha >= 1/255 <=> q <= 2*ln(255)). The per-tile depth-ordered Gaussian
lists are packed into 6 sweeps of exactly <=128 rows each; a tile's list may
SPLIT across two consecutive sweeps — the exclusive cumulative transmittance
of the second part picks up the first part's tail product via an extra
"carry" matmul in log space (carry matrix DMA'd as data, so the device
program stays identical across cores = SPMD).

Device math per sweep (g = packed Gaussian rows, pix = 256 tile-local pixels):
  q[g,pix]   = A[12,g]^T @ B[12,pix]      (PE, f32r hi/lo split, exact basis)
  e          = exp(-q/2)                  (ACT)
  m          = e >= 1/255                 (DVE)
  alpha      = min(e,.99) * m             (DVE fused scalar_tensor_tensor)
  l          = ln(-alpha + 1)             (ACT, free affine does 1-alpha)
  Tlog[g,pix]= StrictLowerBlockDiag @ l   (PE, per-part exclusive cumsum)
               + Carry_s @ l_prev         (PE, split-tile cross-sweep carry)
  T          = exp(Tlog)                  (ACT)
  w          = alpha * T                  (Pool)
  img[group rows] = Colors^T @ w          (PE accumulate over the group's 2
                                           sweeps; DMA'd straight from PSUM)
Host reassembles bands from the per-core (tile, group, row) slot map, adding
the partial sums of split tiles.
"""

import os
import numpy as np

_H = 256
_W = 256
_NCORES = 8
_TS = 16                       # pixel tile edge
_TILES_X = _W // _TS           # 16
_TILES_Y_CORE = (_H // _NCORES) // _TS   # 2 tile rows per core band
_NTILES = _TILES_X * _TILES_Y_CORE       # 32 tiles per core
_NPIX = _TS * _TS              # 256 pixels per tile
_CAP = 128                     # gaussian rows per sweep
_S = 6                         # sweeps (perfect-fill packing w/ splits)
_GROUP = 2                     # sweeps batched per PSUM group
_NG = _S // _GROUP             # groups
_QTH = float(2.0 * np.log(255.0))
_PAD_Q = 100.0                 # q for padding slots -> alpha 0


def _f32r_hi(x):
    """Truncate f32 mantissa to 10 explicit bits (safely representable in
    the PE's reduced-precision f32r streaming format)."""
    xi = np.ascontiguousarray(x, dtype=np.float32).view(np.int32)
    return (xi & np.int32(~0x1FFF)).view(np.float32)


def _reference_numpy(means_2d, covs_2d, depth_features, color_features, H, W):
    """Exact slow fallback (mirrors reference.py math)."""
    order = np.argsort(depth_features, kind="stable")
    m = means_2d[order].astype(np.float32)
    cv = covs_2d[order].astype(np.float32)
    cl = color_features[order].astype(np.float32)
    a, b, c = cv[:, 0], cv[:, 1], cv[:, 2]
    det = a * c - b * b
    ia, ib, ic = c / det, -b / det, a / det
    xs = np.arange(W, dtype=np.float32) + 0.5
    ys = np.arange(H, dtype=np.float32) + 0.5
    img = np.zeros((3, H, W), np.float32)
    T = np.ones((H, W), np.float32)
    for p in range(m.shape[0]):
        dx = xs[None, :] - m[p, 0]
        dy = ys[:, None] - m[p, 1]
        q = ia[p] * dx * dx + 2.0 * ib[p] * dx * dy + ic[p] * dy * dy
        alpha = np.minimum(np.float32(0.99), np.exp(np.float32(-0.5) * q))
        alpha = np.where(alpha < 1.0 / 255.0, np.float32(0.0), alpha)
        w = alpha * T
        img += cl[p][:, None, None] * w[None]
        T = T * (1.0 - alpha)
    return img


def _prep_core(core, m, ia, ib, ic, rx, ry):
    """Per-tile depth-ordered gaussian lists for one core band."""
    tiles = []
    y_base = core * (_H // _NCORES)
    for ty in range(_TILES_Y_CORE):
        y0 = y_base + ty * _TS
        for tx in range(_TILES_X):
            x0 = tx * _TS
            t = ty * _TILES_X + tx
            cand = np.nonzero(
                (m[:, 0] + rx >= x0 + 0.5 - 1e-6)
                & (m[:, 0] - rx <= x0 + _TS - 0.5 + 1e-6)
                & (m[:, 1] + ry >= y0 + 0.5 - 1e-6)
                & (m[:, 1] - ry <= y0 + _TS - 0.5 + 1e-6)
            )[0]
            if cand.size:
                # exact: min over the tile's pixel centers of q <= QTH
                dx = (x0 + 0.5 + np.arange(_TS))[None, :] - m[cand, 0][:, None]
                dy = (y0 + 0.5 + np.arange(_TS))[None, :] - m[cand, 1][:, None]
                q = (
                    ia[cand][:, None, None] * (dx * dx)[:, None, :]
                    + 2.0 * ib[cand][:, None, None]
                    * dx[:, None, :] * dy[:, :, None]
                    + ic[cand][:, None, None] * (dy * dy)[:, :, None]
                )
                qmin = q.reshape(cand.size, -1).min(axis=1)
                cand = cand[qmin <= _QTH + 1e-3]
            tiles.append((t, cand))
    return tiles


def _pack_tiles(tiles):
    """Sequentially fill _S sweeps of exactly _CAP rows, splitting a tile's
    depth-ordered list across two consecutive sweeps when it straddles a
    boundary. Returns parts: list of (tile, sweep, start_row, idx_chunk,
    carry_from) where carry_from = (prev_sweep_start, prev_len) for the
    second half of a split, else None."""
    parts = []
    sweep, cursor = 0, 0
    for t, idx in tiles:
        n = len(idx)
        if n == 0:
            continue
        off = 0
        prev = None
        while n > 0:
            if cursor == _CAP:
                sweep += 1
                cursor = 0
                if sweep >= _S:
                    raise ValueError("packing overflow")
            take = min(n, _CAP - cursor)
            parts.append((t, sweep, cursor, idx[off : off + take], prev))
            prev = (cursor, take)   # for a possible continuation
            cursor += take
            off += take
            n -= take
    return parts


def _build_core_data(core, m, ia, ib, ic, cl, rx, ry, rows_g):
    """Host tensors for one core: A [12, S*128], mask [128, S*128],
    carry [128, (S-1)*128], colors [128, S*rows_g], and slotmap entries
    (tile, group, rowbase)."""
    tiles = _prep_core(core, m, ia, ib, ic, rx, ry)
    parts = _pack_tiles(tiles)

    A = np.zeros((12, _S * _CAP), np.float32)
    A[5, :] = _PAD_Q            # padding slots: q == _PAD_Q everywhere
    mask = np.zeros((128, _S * _CAP), np.float32)
    # carry blocks 0..S-2 = cross-sweep continuation matrices; block S-1 =
    # identity (adds ln(alpha) into the last group's Tlog for the fused
    # w = exp(Tlog + ln alpha) path)
    carry = np.zeros((128, _S * _CAP), np.float32)
    carry[:, (_S - 1) * _CAP :] = np.eye(_CAP, dtype=np.float32)
    colors = np.zeros((128, _S * rows_g), np.float32)
    slotmap = []

    # assign output rows per group: parts in sweeps 2g, 2g+1 get 3 rows each
    grow = [0] * _NG
    y_base = core * (_H // _NCORES)
    for t, s, start, idx, prev in parts:
        g = s // _GROUP
        rowbase = grow[g]
        grow[g] += 3
        if grow[g] > rows_g:
            raise ValueError("group row overflow")
        slotmap.append((t, g, rowbase))
        n = len(idx)
        ty, tx = divmod(t, _TILES_X)
        cx = tx * _TS + 8.0            # tile-local frame origin
        cy = y_base + ty * _TS + 8.0
        mxl = m[idx, 0] - cx
        myl = m[idx, 1] - cy
        g_ia, g_ib, g_ic = ia[idx], ib[idx], ic[idx]
        coef = np.stack(
            [
                g_ia,
                2.0 * g_ib,
                g_ic,
                -2.0 * (g_ia * mxl + g_ib * myl),
                -2.0 * (g_ib * mxl + g_ic * myl),
                g_ia * mxl * mxl + 2.0 * g_ib * mxl * myl
                + g_ic * myl * myl,
            ],
            axis=0,
        )  # [6, n] float64
        hi = _f32r_hi(coef.astype(np.float32))
        lo = (coef - hi.astype(np.float64)).astype(np.float32)
        c0 = s * _CAP + start
        A[:6, c0 : c0 + n] = hi
        A[6:, c0 : c0 + n] = lo
        # mask[row g', col g] = 1 iff g' < g within the part
        mask[start : start + n, c0 : c0 + n] = np.triu(
            np.ones((n, n), np.float32), 1
        )
        if prev is not None:
            # continuation: add the whole part-1 log-sum to every row
            ps, pn = prev
            carry[ps : ps + pn, (s - 1) * _CAP + start : (s - 1) * _CAP + start + n] = 1.0
        colors[start : start + n, s * rows_g + rowbase : s * rows_g + rowbase + 3] = cl[idx]
    return A, mask, carry, colors, slotmap


def _count_rows_g(m, ia, ib, ic, rx, ry):
    """Max output rows needed by any group on any core (3 per tile-part)."""
    worst = 0
    for core in range(_NCORES):
        parts = _pack_tiles(_prep_core(core, m, ia, ib, ic, rx, ry))
        grow = [0] * _NG
        for t, s, start, idx, prev in parts:
            grow[s // _GROUP] += 3
        worst = max(worst, max(grow))
    return worst


def _basis():
    lc = np.arange(_TS, dtype=np.float32) - 7.5
    xl = np.tile(lc, _TS)                     # pixel p = lr*16+lc
    yl = np.repeat(lc, _TS)
    B = np.stack(
        [xl * xl, xl * yl, yl * yl, xl, yl, np.ones(_NPIX, np.float32)], 0
    )
    return np.concatenate([B, B], axis=0).astype(np.float32)   # [12, 256]


def _build_program(rows_g, reps=1):
    from contextlib import ExitStack

    import concourse.bacc as bacc
    import concourse.hw_specs as hw_specs
    import concourse.tile as tile
    from concourse import mybir

    F32 = mybir.dt.float32
    F32R = mybir.dt.float32r
    AF = mybir.ActivationFunctionType
    OP = mybir.AluOpType
    S = _S

    # Our kernel alternates Exp and Ln; make sure the act-table chooser can
    # only satisfy both from the combined set (one table load instead of a
    # ~1.3us reload per switch). Keys and their order are preserved so the
    # emitted act_func_set_id indices stay aligned with act_info.json.
    if not getattr(hw_specs, "_gs_act_patch", False):
        _orig_get_tables = hw_specs.get_activation_tables

        def _patched(arch):
            tables = _orig_get_tables(arch)
            for name, funcs in tables.items():
                if name != "natural_log_exp_and_others":
                    funcs.discard(mybir.ActivationFunctionType.Exp)
                    funcs.discard(mybir.ActivationFunctionType.Ln)
            return tables

        hw_specs.get_activation_tables = _patched
        bacc.get_activation_tables = _patched
        hw_specs._gs_act_patch = True

    nc = bacc.Bacc(trn_type="TRN2", target_bir_lowering=False, debug=False)
    t_A = nc.dram_tensor(
        "A", [12, _NPIX + S * _CAP], F32, kind="ExternalInput"
    )  # basis in cols [0, _NPIX), coefficients after
    t_mask = nc.dram_tensor("maskl", [128, S * _CAP], F32, kind="ExternalInput")
    t_carry = nc.dram_tensor(
        "carry", [128, S * _CAP], F32, kind="ExternalInput"
    )
    t_col = nc.dram_tensor(
        "colors", [128, S * rows_g], F32, kind="ExternalInput"
    )
    t_out = nc.dram_tensor(
        "out", [_NG * rows_g, _NPIX], F32, kind="ExternalOutput"
    )

    GW = _GROUP * _NPIX        # group free width

    with ExitStack() as ctx:
        tc = ctx.enter_context(tile.TileContext(nc))
        const = ctx.enter_context(tc.tile_pool(name="const", bufs=1))
        sb = ctx.enter_context(tc.tile_pool(name="sb", bufs=4))
        psq = ctx.enter_context(tc.tile_pool(name="psq", bufs=2, space="PSUM"))
        pst = ctx.enter_context(tc.tile_pool(name="pst", bufs=2, space="PSUM"))
        psi = ctx.enter_context(tc.tile_pool(name="psi", bufs=2, space="PSUM"))

        AB_all = const.tile([12, _NPIX + S * _CAP], F32)
        mask_all = const.tile([128, S * _CAP], F32)
        carry_all = const.tile([128, S * _CAP], F32)
        col_all = const.tile([128, S * rows_g], F32)

        # basis + A coefficients on the SP queue (gate the q matmuls;
        # chunk 0 carries basis + group-0 coeffs in one DMA so only one
        # DMA completion latency sits before the first matmul);
        # masks on the gpsimd queue in parallel; carries on the vector
        # queue (first chunk alone so sweep-1's carry lands early).
        CW = _GROUP * _CAP
        nc.gpsimd.dma_start(
            AB_all[:, : _NPIX + CW].bitcast(F32R),
            t_A[:, : _NPIX + CW].bitcast(F32R),
        )
        c0 = _NPIX + CW
        nc.sync.dma_start(
            AB_all[:, c0 : c0 + CW].bitcast(F32R),
            t_A[:, c0 : c0 + CW].bitcast(F32R),
        )
        nc.sync.dma_start(
            carry_all[:, :_CAP].bitcast(F32R), t_carry[:, :_CAP].bitcast(F32R)
        )
        c0 = _NPIX + 2 * CW
        nc.sync.dma_start(
            AB_all[:, c0 : c0 + CW].bitcast(F32R),
            t_A[:, c0 : c0 + CW].bitcast(F32R),
        )
        nc.sync.dma_start(
            carry_all[:, _CAP:].bitcast(F32R), t_carry[:, _CAP:].bitcast(F32R)
        )
        for g in range(_NG):
            nc.gpsimd.dma_start(
                mask_all[:, g * CW : (g + 1) * CW].bitcast(F32R),
                t_mask[:, g * CW : (g + 1) * CW].bitcast(F32R),
            )
        nc.gpsimd.dma_start(col_all[:].bitcast(F32R), t_col[:].bitcast(F32R))

        basis = AB_all[:, :_NPIX]
        A_t = [
            AB_all[:, _NPIX + s * _CAP : _NPIX + (s + 1) * _CAP]
            for s in range(S)
        ]
        mask_t = [mask_all[:, s * _CAP : (s + 1) * _CAP] for s in range(S)]
        carry_t = [carry_all[:, s * _CAP : (s + 1) * _CAP] for s in range(S)]
        ident = carry_t[S - 1]
        col_t = [col_all[:, s * rows_g : (s + 1) * rows_g] for s in range(S)]

        # warm the PE clock (HAM) while input DMAs are in flight
        psw = ctx.enter_context(tc.tile_pool(name="psw", bufs=1, space="PSUM"))
        warm = const.tile([128, 16], F32)
        nc.vector.memset(warm[:], 0.0)
        warm_ps = psw.tile([128, 16], F32)
        for _ in range(14):
            nc.tensor.matmul(
                warm_ps[:16, :16], warm[:], warm[:, :16], start=True, stop=True
            )

        l_tiles = {}
        for g in range(_NG * reps):
            g = g % _NG
            q4 = psq.tile([128, GW], F32)
            for i in range(_GROUP):
                s = g * _GROUP + i
                nc.tensor.matmul(
                    q4[:, i * _NPIX : (i + 1) * _NPIX],
                    A_t[s].bitcast(F32R),
                    basis.bitcast(F32R),
                    start=True,
                    stop=True,
                )
            last = g == _NG - 1
            e4 = sb.tile([128, GW], F32, tag="e")
            nc.scalar.activation(e4[:], q4[:], AF.Exp, scale=-0.5)
            m4 = sb.tile([128, GW], F32, tag="m")
            if last:
                # keep the DVE free for al/u on the critical Ln path
                nc.gpsimd.tensor_scalar(
                    m4[:], e4[:], 1.0 / 255.0, None, OP.is_ge
                )
            else:
                nc.vector.tensor_scalar(
                    m4[:], e4[:], 1.0 / 255.0, None, OP.is_ge
                )
            al4 = sb.tile([128, GW], F32, tag="al")
            nc.vector.scalar_tensor_tensor(
                al4[:], e4[:], 0.99, m4[:], OP.min, OP.mult
            )
            if last:
                # ln(alpha) sans 1/255 cutoff = -0.5*max(q, -2 ln .99);
                # fused w = exp(Tlog + ln alpha) skips the Pool multiply on
                # the critical tail (sub-threshold w error ~4e-3 rel, in
                # budget)
                u4 = sb.tile([128, GW], F32, tag="u")
                nc.vector.tensor_scalar(
                    u4[:].bitcast(F32R),
                    q4[:],
                    0.02010067170299715,
                    -0.5,
                    OP.max,
                    OP.mult,
                )
            l4 = sb.tile([128, GW], F32, tag=f"l{g % 2}")
            l_tiles[g] = l4
            nc.scalar.activation(
                l4[:].bitcast(F32R), al4[:], AF.Ln, bias=1.0, scale=-1.0
            )
            tl4 = pst.tile([128, GW], F32)
            for i in range(_GROUP):
                s = g * _GROUP + i
                dst = tl4[:, i * _NPIX : (i + 1) * _NPIX]
                if last:
                    nc.tensor.matmul(
                        dst,
                        ident.bitcast(F32R),
                        u4[:, i * _NPIX : (i + 1) * _NPIX].bitcast(F32R),
                        start=True,
                        stop=False,
                    )
                nc.tensor.matmul(
                    dst,
                    mask_t[s].bitcast(F32R),
                    l4[:, i * _NPIX : (i + 1) * _NPIX].bitcast(F32R),
                    start=not last,
                    stop=(s == 0),
                )
                if s > 0:
                    lprev = (
                        l4[:, :_NPIX]
                        if i == 1
                        else l_tiles[g - 1][:, _NPIX:]
                    )
                    nc.tensor.matmul(
                        dst,
                        carry_t[s - 1].bitcast(F32R),
                        lprev.bitcast(F32R),
                        start=False,
                        stop=True,
                    )
            if last:
                w4 = sb.tile([128, GW], F32, tag="w")
                nc.scalar.activation(w4[:], tl4[:], AF.Exp)
            else:
                T4 = sb.tile([128, GW], F32, tag="T")
                nc.scalar.activation(T4[:], tl4[:], AF.Exp)
                w4 = sb.tile([128, GW], F32, tag="w")
                nc.gpsimd.tensor_tensor(
                    w4[:].bitcast(F32R), al4[:], T4[:], OP.mult
                )
            img = psi.tile([rows_g, _NPIX], F32, tag="img", name="img")
            for i in range(_GROUP):
                s = g * _GROUP + i
                nc.tensor.matmul(
                    img[:],
                    col_t[s].bitcast(F32R),
                    w4[:, i * _NPIX : (i + 1) * _NPIX].bitcast(F32R),
                    start=(i == 0),
                    stop=(i == _GROUP - 1),
                )
            out_sb = sb.tile([rows_g, _NPIX], F32, tag="osb", name="osb")
            nc.vector.tensor_copy(out_sb[:], img[:])
            nc.sync.dma_start(
                t_out[g * rows_g : (g + 1) * rows_g, :], out_sb[:]
            )

    nc.compile()
    return nc


def kernel(means_2d, covs_2d, depth_features, color_features, height, width):
    H, W = int(height), int(width)
    means_2d = np.asarray(means_2d, np.float32)
    covs_2d = np.asarray(covs_2d, np.float32)
    depth_features = np.asarray(depth_features, np.float32)
    color_features = np.asarray(color_features, np.float32)

    a, b, c = (
        covs_2d[:, 0].astype(np.float64),
        covs_2d[:, 1].astype(np.float64),
        covs_2d[:, 2].astype(np.float64),
    )
    det = a * c - b * b
    if H != _H or W != _W or np.any(det <= 0) or np.any(a <= 0) or np.any(c <= 0):
        return _reference_numpy(
            means_2d, covs_2d, depth_features, color_features, H, W
        )

    order = np.argsort(depth_features, kind="stable")
    m = means_2d[order].astype(np.float64)
    cvo = covs_2d[order].astype(np.float64)
    cl = color_features[order].astype(np.float32)
    a, b, c = cvo[:, 0], cvo[:, 1], cvo[:, 2]
    det = a * c - b * b
    ia, ib, ic = c / det, -b / det, a / det
    rx = np.sqrt(_QTH * a) + 1e-3
    ry = np.sqrt(_QTH * c) + 1e-3

    try:
        rows_g = _count_rows_g(m, ia, ib, ic, rx, ry)
        in_maps = []
        slotmaps = []
        basis = _basis()
        for core in range(_NCORES):
            A, mask, carry, colors, slotmap = _build_core_data(
                core, m, ia, ib, ic, cl, rx, ry, rows_g
            )
            in_maps.append(
                {
                    "A": np.ascontiguousarray(
                        np.concatenate([basis, A], axis=1)
                    ),
                    "maskl": mask,
                    "carry": carry,
                    "colors": colors,
                }
            )
            slotmaps.append(slotmap)
    except ValueError:
        return _reference_numpy(
            means_2d, covs_2d, depth_features, color_features, H, W
        )

    nc = _build_program(rows_g)
    if os.environ.get("GS_KERNEL_SIM") == "1":
        from types import SimpleNamespace

        from concourse.bass_interp import CoreSim

        results = []
        for core in range(_NCORES):
            sim = CoreSim(nc)
            for k, v in in_maps[core].items():
                sim.tensor(k)[:] = v
            sim.simulate()
            results.append({"out": np.array(sim.tensor("out"))})
        res = SimpleNamespace(results=results)
    else:
        from concourse.bass_utils import run_bass_kernel_spmd

        res = run_bass_kernel_spmd(nc, in_maps, core_ids=list(range(_NCORES)))

    img = np.zeros((3, _H, _W), np.float32)
    band = _H // _NCORES
    for core in range(_NCORES):
        o = res.results[core]["out"]  # [_NG*rows_g, 256]
        for t, g, rowbase in slotmaps[core]:
            ty, tx = divmod(t, _TILES_X)
            blk = o[g * rows_g + rowbase : g * rows_g + rowbase + 3].reshape(
                3, _TS, _TS
            )
            img[
                :,
                core * band + ty * _TS : core * band + (ty + 1) * _TS,
                tx * _TS : (tx + 1) * _TS,
            ] += blk
    return img
